# revision 1
# baseline (speedup 1.0000x reference)
"""CapsuleLinear (dynamic routing) Trainium2 kernel.

Reference computes priors = einsum('oli,bni->bonl', W, x) (302MB) then runs 3
routing iterations. We never materialize priors; per routing iteration:
    probs[n,o]   = softmax_o(logits[n,o])              (exp on ACT, Z on DVE)
    s[o,i]       = sum_n probs[n,o] * x[n,i]           (PE matmul, contract n)
    out[o,l]     = sum_i W[o,l,i] * s[o,i]             (DVE/GPSIMD mul+reduce)
    v            = squash(out)
    wv[o,i]      = sum_l W[o,l,i] * v[o,l]             (mul+reduce)
    logits[n,o] += sum_i x[n,i] * wv[o,i]              (PE matmul, contract i,
                                                        accumulates in PSUM)
Sharding: data-parallel over batch N=32 -> 4 batches per core on 8 cores.
Weight (64,32,32) replicated. No collectives.

Matmul operands are bf16 (measured end-to-end rel err ~5e-3; PSUM accumulation
stays fp32); the capsule-vector path (out-step, squash, wv) stays fp32.
sqrt(ns) is computed as exp(0.5*ln(ns)) so the whole kernel uses one ACT
table set (natural_log_exp_and_others) - no 1.3us table switches.

Per-core layouts:
  x_sb  [128(p), 4(b), 9(c), 32(i)]   x[b, c*128+p, i]          bf16
  xt_sb [32(i), 4(b), 9(c), 128(p)]   host-transposed x         bf16
  w_li  [128(b2*64+o), 32(l), 32(i)]  W pair-replicated         fp32
  w_il  [128(b2*64+o), 32(i), 32(l)]                            fp32
  logits PSUM [128(p), 4(b), 9(c), 64(o)] resident, fp32
  pair tiles [128(b2*64+o), 2(pair), ...] 2 batches stacked on partitions
"""

import os
import sys

for _p in ("/opt/trn_rl_repo",):
    if _p not in sys.path and os.path.isdir(_p):
        sys.path.insert(0, _p)

import numpy as np

import concourse.bacc as bacc
import concourse.bass as bass
import concourse.tile as tile
from concourse import mybir
from concourse.bass_utils import run_bass_kernel_spmd

CFG_BF16 = os.environ.get("K_BF16", "1") == "1"
CFG_TTR = os.environ.get("K_TTR", "0") == "1"  # TensorTensorReduce hangs TRN2 HW here
CFG_LNEXP = os.environ.get("K_LNEXP", "1") == "1"
CFG_GPS = os.environ.get("K_GPS", "1") == "1"
CFG_WBF = os.environ.get("K_WBF", "0") == "1"   # bf16 W / s / prod path (off: keeps rel err ~6e-3)
CFG_DBF = os.environ.get("K_DBF", "1") == "1"   # bf16 delta matmul (xt/wvT)
CFG_SQACC = os.environ.get("K_SQACC", "1") == "1"  # ns via ACT Square+accum

N_TOT, N_CAPS, I_LEN = 32, 1152, 32
O_CAPS, L_LEN = 64, 32
NCORES = 8
B = N_TOT // NCORES  # 4 batches per core
C = N_CAPS // 128    # 9 chunks of 128 input capsules
PAIRS = B // 2
FP = mybir.dt.float32
BF = mybir.dt.bfloat16
Exp = mybir.ActivationFunctionType.Exp
Ln = mybir.ActivationFunctionType.Ln
Square = mybir.ActivationFunctionType.Square
X = mybir.AxisListType.X
MUL = mybir.AluOpType.mult
BD = None  # set below: bf16 matmul-operand dtype, or fp32 when disabled


def build_nc():
    nc = bacc.Bacc("TRN2", target_bir_lowering=False, debug=True)
    BD = BF if CFG_BF16 else FP
    WD = BF if CFG_WBF else FP
    DD = BF if (CFG_BF16 and CFG_DBF) else FP
    x_nat_d = nc.dram_tensor("x_nat", [128, B, C, I_LEN], BD, kind="ExternalInput")
    xt_d = nc.dram_tensor("xt", [I_LEN, B, C, 128], DD, kind="ExternalInput")
    w_li_d = nc.dram_tensor("w_li", [128, L_LEN, I_LEN], WD, kind="ExternalInput")
    w_il_d = nc.dram_tensor("w_il", [128, I_LEN, L_LEN], WD, kind="ExternalInput")
    ident_d = nc.dram_tensor("ident", [128, 128], FP, kind="ExternalInput")
    out_d = nc.dram_tensor("out", [PAIRS, 128, L_LEN], FP, kind="ExternalOutput")

    with tile.TileContext(nc) as tc:
        with (
            tc.tile_pool(name="main", bufs=1) as pool,
            tc.tile_pool(name="psum", bufs=1, space="PSUM") as psum,
        ):
            x_sb = pool.tile([128, B, C, I_LEN], BD)
            xt_sb = pool.tile([I_LEN, B, C, 128], DD)
            wli_sb = pool.tile([128, L_LEN, I_LEN], WD)
            wil_sb = pool.tile([128, I_LEN, L_LEN], WD)
            ident = pool.tile([128, 128], FP)
            ones64 = pool.tile([128, O_CAPS], BD)
            shift = pool.tile([128, 1], FP)
            pexp = pool.tile([128, B, C, O_CAPS], BD)
            zsum = pool.tile([128, B, C], FP)
            rinv = pool.tile([128, B, C], FP)
            xr = pool.tile([128, B, C, I_LEN], BD)
            s_sb = pool.tile([128, PAIRS, I_LEN], WD)
            prod = pool.tile([128, PAIRS, L_LEN, I_LEN], WD)
            v_raw = pool.tile([128, PAIRS, L_LEN], FP)
            sq = pool.tile([128, PAIRS, L_LEN], FP)
            ns = pool.tile([128, PAIRS], FP)
            lnns = pool.tile([128, PAIRS], FP)
            vnorm = pool.tile([128, PAIRS], FP)
            denom = pool.tile([128, PAIRS], FP)
            rden = pool.tile([128, PAIRS], FP)
            factor = pool.tile([128, PAIRS], FP)
            v = pool.tile([128, PAIRS, L_LEN], FP)
            v_bf = pool.tile([128, PAIRS, L_LEN], WD)
            wprod = pool.tile([128, PAIRS, I_LEN, L_LEN], WD)
            wv = pool.tile([128, PAIRS, I_LEN], FP)
            wvt_sb = pool.tile([I_LEN, PAIRS, 128], DD)

            # logits PSUM, split into two 2-batch tiles so an iteration's
            # exp(b) only waits on its own half's delta matmuls. 18 chunks of
            # 256B per tile -> 2.25 banks (padded to 3). A matmul with
            # start=True lazily zeroes its whole bank, so emit start only on
            # the first chunk of each bank (r=0) and stop on the last.
            logits_ps = [
                psum.tile([128, 2, C, O_CAPS], FP, name=f"logits_ps{h}", tag=f"lg{h}")
                for h in range(2)
            ]
            # s (bytes 0..127) and wvT (bytes 512..1023) share a bank per pair;
            # the s -> v -> wv -> wvT dependency chain orders their lifetimes.
            u_ps = [
                psum.tile([128, 512], FP, name=f"u_ps{t}", tag=f"u_ps{t}")
                for t in range(PAIRS)
            ]
            s_ps = [u_ps[t][:, 0:I_LEN] for t in range(PAIRS)]
            wvt_ps = [u_ps[t][0:I_LEN, 128:256] for t in range(PAIRS)]

            dma = nc.sync
            # split/spread input DMAs across the three DMA-capable queues
            # (sync/scalar/gpsimd) in consumption order: x feeds the first
            # matmuls, w_li the out-step ~1us later, then w_il/ident/xt.
            # w_li/w_il are pair-replicated: fetch 64 rows from HBM, then a
            # local SBUF->SBUF DMA fills rows 64..127 (halves HBM traffic).
            nc.scalar.dma_start(out=wli_sb[0:64], in_=w_li_d[0:64])
            for b in range(B):
                dma.dma_start(out=x_sb[:, b], in_=x_nat_d[:, b])
            nc.scalar.dma_start(out=wil_sb[0:64], in_=w_il_d[0:64])
            nc.scalar.dma_start(out=wli_sb[64:128], in_=wli_sb[0:64])
            nc.scalar.dma_start(out=wil_sb[64:128], in_=wil_sb[0:64])
            nc.gpsimd.dma_start(out=ident[:], in_=ident_d[:])
            nc.gpsimd.dma_start(out=xt_sb[:], in_=xt_d[:])
            nc.vector.memset(ones64[:], 1.0)
            nc.vector.memset(shift[:], -40.0)

            for r in range(3):
                for b in range(B):
                    t, b2 = divmod(b, 2)
                    if r > 0:
                        # softmax numerator & partition function, per batch so
                        # the exp->Z->1/Z->xr->matmul chain pipelines over b.
                        # exp(l - 40): softmax-invariant shift keeps exp and
                        # 1/Z in fp32 range (logits span [-86, 92] here).
                        nc.scalar.activation(
                            out=pexp[:, b], in_=logits_ps[b // 2][:, b % 2],
                            func=Exp, bias=shift[:],
                        )
                        nc.vector.reduce_sum(out=zsum[:, b], in_=pexp[:, b], axis=X)
                        nc.vector.reciprocal(out=rinv[:, b], in_=zsum[:, b])
                        (nc.gpsimd if CFG_GPS else nc.vector).tensor_mul(
                            out=xr[:, b],
                            in0=x_sb[:, b],
                            in1=rinv[:, b].unsqueeze(-1).broadcast_to((128, C, I_LEN)),
                        )
                    # s[o,i] = sum_n probs * x  (iter 0: probs uniform -> ones)
                    for c in range(C):
                        nc.tensor.matmul(
                            out=s_ps[t][b2 * 64 : (b2 + 1) * 64, :],
                            lhsT=ones64[:] if r == 0 else pexp[:, b, c, :],
                            rhs=x_sb[:, b, c, :] if r == 0 else xr[:, b, c, :],
                            start=(c == 0),
                            stop=(c == C - 1),
                            tile_position=(0, 64 * b2),
                        )
                # PSUM -> SBUF (fold the uniform 1/64 prob into iter-0 copy)
                for t in range(PAIRS):
                    nc.scalar.mul(
                        out=s_sb[:, t, :],
                        in_=s_ps[t][:],
                        mul=(1.0 / 64 if r == 0 else 1.0),
                    )
                # out[o,l] = sum_i W[o,l,i] * s[o,i]; muls split DVE/GPSIMD
                for t in range(PAIRS):
                    nc.vector.tensor_mul(
                        out=prod[:, t],
                        in0=wli_sb[:],
                        in1=s_sb[:, t, :].unsqueeze(1).broadcast_to((128, L_LEN, I_LEN)),
                    )
                    nc.vector.reduce_sum(out=v_raw[:, t, :], in_=prod[:, t], axis=X)
                    # squash: factor = ||v||/(1+||v||^2); ns via fused TTR
                    if CFG_SQACC:
                        nc.scalar.activation(
                            out=sq[:, t],
                            in_=v_raw[:, t],
                            func=Square,
                            accum_out=ns[:, t : t + 1],
                        )
                    else:
                        nc.vector.tensor_mul(out=sq[:, t], in0=v_raw[:, t], in1=v_raw[:, t])
                        nc.vector.reduce_sum(out=ns[:, t : t + 1], in_=sq[:, t].unsqueeze(1), axis=X)
                # sqrt(ns) = exp(0.5*ln(ns)): stays in one ACT table set.
                # All squash ops split per pair so pair0's wv/delta chain
                # never waits on pair1's reduce.
                for t in range(PAIRS):
                    tsl = slice(t, t + 1)
                    if CFG_LNEXP:
                        nc.scalar.activation(out=lnns[:, tsl], in_=ns[:, tsl], func=Ln)
                        nc.scalar.activation(
                            out=vnorm[:, tsl], in_=lnns[:, tsl], func=Exp, scale=0.5
                        )
                    else:
                        nc.scalar.sqrt(out=vnorm[:, tsl], in_=ns[:, tsl])
                    nc.vector.tensor_scalar_add(
                        out=denom[:, tsl], in0=ns[:, tsl], scalar1=1.0
                    )
                    nc.vector.reciprocal(out=rden[:, tsl], in_=denom[:, tsl])
                    # v = (v_raw * ||v||) * (1/(1+||v||^2)) fused in one op
                    nc.vector.scalar_tensor_tensor(
                        out=(v[:, t] if r == 2 else v_bf[:, t]),
                        in0=v_raw[:, t],
                        scalar=vnorm[:, tsl],
                        in1=rden[:, tsl].broadcast_to((128, L_LEN)),
                        op0=MUL,
                        op1=MUL,
                    )
                if r == 2:
                    for t in range(PAIRS):
                        dma.dma_start(out=out_d[t], in_=v[:, t, :])
                else:
                    # wv[o,i] = sum_l W[o,l,i] * v[o,l]
                    for t in range(PAIRS):
                        nc.vector.tensor_mul(
                            out=wprod[:, t],
                            in0=wil_sb[:],
                            in1=v_bf[:, t, :]
                            .unsqueeze(1)
                            .broadcast_to((128, I_LEN, L_LEN)),
                        )
                        nc.vector.reduce_sum(out=wv[:, t, :], in_=wprod[:, t], axis=X)
                        nc.tensor.transpose(
                            out=wvt_ps[t][:], in_=wv[:, t, :], identity=ident[:]
                        )
                        nc.scalar.copy(out=wvt_sb[:, t, :], in_=wvt_ps[t][:])
                    # logits[n,o] += sum_i x[n,i] * wv[o,i]
                    # r0: one start/stop per 2KB psum bank (8 chunks per bank).
                    # r1: accumulate onto surviving has_written bits; the sim's
                    # group bookkeeping can't express re-opening, so skip it.
                    for b in range(B):
                        t, b2 = divmod(b, 2)
                        for c in range(C):
                            k = (b % 2) * C + c
                            nc.tensor.matmul(
                                out=logits_ps[b // 2][:, b % 2, c, :],
                                lhsT=xt_sb[:, b, c, :],
                                rhs=wvt_sb[:, t, b2 * 64 : (b2 + 1) * 64],
                                start=(r == 0 and k % 8 == 0),
                                stop=(r == 0 and (k % 8 == 7 or k == 2 * C - 1)),
                                skip_group_check=(r == 1),
                            )
    return nc


_NC = None


def get_nc():
    global _NC
    if _NC is None:
        _NC = build_nc()
    return _NC


def make_in_maps(x, weight):
    x = np.ascontiguousarray(x, dtype=np.float32)
    w = np.ascontiguousarray(weight, dtype=np.float32)
    w_li = np.tile(w.reshape(O_CAPS, L_LEN, I_LEN), (2, 1, 1))
    w_il = np.tile(w.transpose(0, 2, 1), (2, 1, 1))
    ident = np.eye(128, dtype=np.float32)
    in_maps = []
    for core in range(NCORES):
        xs = x[core * B : (core + 1) * B]  # [B, 1152, 32]
        xc = xs.reshape(B, C, 128, I_LEN)
        x_nat = np.ascontiguousarray(xc.transpose(2, 0, 1, 3))  # [128, B, C, 32]
        xt = np.ascontiguousarray(xc.transpose(3, 0, 1, 2))  # [32, B, C, 128]
        in_maps.append(
            {
                "x_nat": to_bf16(x_nat) if CFG_BF16 else x_nat,
                "xt": to_bf16(xt) if (CFG_BF16 and CFG_DBF) else xt,
                "w_li": to_bf16(w_li) if CFG_WBF else w_li,
                "w_il": to_bf16(w_il) if CFG_WBF else w_il,
                "ident": ident,
            }
        )
    return in_maps


def to_bf16(a):
    import ml_dtypes

    return a.astype(ml_dtypes.bfloat16)


def assemble(results):
    outs = []
    for core in range(NCORES):
        o = results[core]["out"]  # [PAIRS, 128, 32] -> [4, 64, 32]
        outs.append(np.asarray(o, dtype=np.float32).reshape(B, O_CAPS, L_LEN))
    return np.concatenate(outs, axis=0)


def _pin_act_table_set(nc):
    """Make Exp and Ln resolve to the one table set containing both
    (natural_log_exp_and_others), so the whole kernel runs on a single
    ACT table load instead of thrashing 1.3us loads between exp/ln sets.
    Mutates the cached dict in place; set indices stay aligned with
    act_info.json."""
    from concourse.hw_specs import get_activation_tables

    tabs = get_activation_tables(nc.m.arch)
    for name, funcs in tabs.items():
        if name != "natural_log_exp_and_others":
            funcs.discard(Exp)
            funcs.discard(Ln)
            funcs.discard(Square)
            funcs.discard(mybir.ActivationFunctionType.Copy)
            funcs.discard(mybir.ActivationFunctionType.Identity)


def run(x, weight, trace=False):
    nc = get_nc()
    if not nc.is_finalized():
        _pin_act_table_set(nc)
        nc.finalize()  # run Bacc lowering passes (wait splitting, reg alloc)
    res = run_bass_kernel_spmd(nc, make_in_maps(x, weight), list(range(NCORES)), trace=trace)
    return assemble(res.results), res


def kernel(x, weight):
    out, _ = run(x, weight)
    return out



# revision 7
# speedup vs baseline: 1.1585x; 1.1585x over previous
"""CapsuleLinear (dynamic routing) Trainium2 kernel, v2.

Reference computes priors = einsum('oli,bni->bonl', W, x) (302MB) then runs 3
routing iterations. We never materialize priors; per routing iteration:
    probs[n,o]   = softmax_o(logits[n,o])              (exp on ACT, Z on DVE)
    s[o,i]       = sum_n probs[n,o] * x[n,i]           (PE matmul, contract n)
    v_raw[o,l]   = sum_i W[o,l,i] * s[o,i]             (DVE bf16 mul+reduce)
    f[o]         = ||v_raw|| / (1 + ||v_raw||^2)       (squash factor)
    wv[o,i]      = f * sum_l W[o,l,i] * v_raw[o,l]     (DVE bf16; f decoupled)
    logits[n,o] += sum_i x[n,i] * wv[o,i]              (PE matmul, contract i)

v2 structural changes vs v1:
  - iteration 0's s is a plain column sum of x (probs uniform): computed on
    the host (0.001% of FLOPs) and shipped as a 16KB input, so the iter-0
    capsule step + wv + transpose overlap the big x/xt DMAs; the first thing
    that waits on x is the iter-0 delta matmul.
  - the out-step / wv-step run fully in bf16 on DVE (2-4x throughput modes),
    pair tiles [128 = 2 batches x 64 o, 32, 32].
  - the squash scalar chain (ns -> ln -> exp -> 1/(1+ns)) runs in parallel
    with the wv_raw tensor contraction; the scalar factor f is applied to
    wv_raw afterwards (wv is linear in v, so f commutes out).
  - consolidated DMAs: one per input tensor, W pair-replication via local
    SBUF->SBUF DMA of rows 0..63 -> 64..127.

Sharding: data-parallel over batch N=32 -> 4 batches per core on 8 cores.
Weight (64,32,32) replicated. No collectives.

sqrt(ns) is computed as exp(0.5*ln(ns)) so the whole kernel uses one ACT
table set (natural_log_exp_and_others) - no 1.3us table switches.

Per-core layouts:
  x_sb  [128(p), 4(b), 9(c), 32(i)]   x[b, c*128+p, i]          bf16
  xt_sb [32(i), 4(b), 9(c), 128(p)]   host-transposed x         bf16
  wli   [128(b2*64+o), 32(l), 32(i)]  W pair-replicated         bf16
  wil   [128(b2*64+o), 32(i), 32(l)]                            bf16
  s0    [128(b2*64+o), 2(t), 32(i)]   host colsum(x)/64         bf16
  logits PSUM [128(p), 2(b2), 9(c), 64(o)] x2 halves, fp32
  pair tiles [128(b2*64+o), 2(pair), ...] 2 batches stacked on partitions
"""

import os
import sys

for _p in ("/opt/trn_rl_repo",):
    if _p not in sys.path and os.path.isdir(_p):
        sys.path.insert(0, _p)

import numpy as np

import concourse.bacc as bacc
import concourse.bass as bass
import concourse.tile as tile
from concourse import mybir
from concourse.bass_utils import run_bass_kernel_spmd

N_TOT, N_CAPS, I_LEN = 32, 1152, 32
O_CAPS, L_LEN = 64, 32
NCORES = 8
B = N_TOT // NCORES  # 4 batches per core
C = N_CAPS // 128    # 9 chunks of 128 input capsules
PAIRS = B // 2
FP = mybir.dt.float32
BF = mybir.dt.bfloat16
Exp = mybir.ActivationFunctionType.Exp
Ln = mybir.ActivationFunctionType.Ln
X = mybir.AxisListType.X
MUL = mybir.AluOpType.mult

# Env toggles for A/B experiments
CFG_ZGPS = os.environ.get("K_ZGPS", "0") == "1"   # Z-reduce on gpsimd
CFG_XRDVE = os.environ.get("K_XRDVE", "0") == "1"  # xr mul on DVE instead of gpsimd


def build_nc():
    nc = bacc.Bacc("TRN2", target_bir_lowering=False, debug=True)
    x_nat_d = nc.dram_tensor("x_nat", [128, B, C, I_LEN], BF, kind="ExternalInput")
    xt_d = nc.dram_tensor("xt", [I_LEN, B, C, 128], BF, kind="ExternalInput")
    # wpack rows 0..63: [wli (32*32) | wil (32*32)] bf16; replicated on-chip
    wpack_d = nc.dram_tensor("wpack", [64, 2048], BF, kind="ExternalInput")
    ident_d = nc.dram_tensor("ident", [128, 128], FP, kind="ExternalInput")
    s0_d = nc.dram_tensor("s0", [128, PAIRS, I_LEN], BF, kind="ExternalInput")
    out_d = nc.dram_tensor("out", [128, PAIRS, L_LEN], FP, kind="ExternalOutput")

    with tile.TileContext(nc) as tc:
        with (
            tc.tile_pool(name="main", bufs=1) as pool,
            tc.tile_pool(name="psum", bufs=1, space="PSUM") as psum,
        ):
            x_sb = pool.tile([128, B, C, I_LEN], BF)
            xt_sb = pool.tile([I_LEN, B, C, 128], BF)
            wpack_sb = pool.tile([128, 2, L_LEN, I_LEN], BF)
            wli_sb = wpack_sb[:, 0]          # [128, 32(l), 32(i)]
            wil_sb = wpack_sb[:, 1]          # [128, 32(i), 32(l)]
            ident = pool.tile([128, 128], FP)
            shift = pool.tile([128, 1], FP)
            s_sb = pool.tile([128, PAIRS, I_LEN], BF)   # s0 DMA lands here too
            pexp = pool.tile([128, B, C, O_CAPS], BF)
            zsum = pool.tile([128, B, C], BF)
            rinv = pool.tile([128, B, C], BF)
            xr = pool.tile([128, B, C, I_LEN], BF)
            prod = pool.tile([128, PAIRS, L_LEN, I_LEN], BF)
            v_raw = pool.tile([128, PAIRS, L_LEN], BF)
            sq = pool.tile([128, PAIRS, L_LEN], BF)
            ns = pool.tile([128, PAIRS], FP)
            lnns = pool.tile([128, PAIRS], FP)
            vnorm = pool.tile([128, PAIRS], FP)
            denom = pool.tile([128, PAIRS], FP)
            rden = pool.tile([128, PAIRS], FP)
            fsc = pool.tile([128, PAIRS], FP)
            wprod = pool.tile([128, PAIRS, I_LEN, L_LEN], BF)
            wv_raw = pool.tile([128, PAIRS, I_LEN], BF)
            wv_f = pool.tile([128, PAIRS, I_LEN], FP)
            wvt_sb = pool.tile([I_LEN, PAIRS, 128], BF)
            v_out = pool.tile([128, PAIRS, L_LEN], FP)

            # logits PSUM, split into two 2-batch tiles so an iteration's
            # exp(b) only waits on its own half's delta matmuls. A matmul
            # with start=True lazily zeroes its whole bank, so emit start
            # only on the first chunk of each bank and stop on the last.
            logits_ps = [
                psum.tile([128, 2, C, O_CAPS], FP, name=f"logits_ps{h}", tag=f"lg{h}")
                for h in range(2)
            ]
            # s (bytes 0..127) and wvT (bytes 512..1023) share a bank per pair;
            # the s -> v -> wv -> wvT dependency chain orders their lifetimes.
            u_ps = [
                psum.tile([128, 512], FP, name=f"u_ps{t}", tag=f"u_ps{t}")
                for t in range(PAIRS)
            ]
            s_ps = [u_ps[t][:, 0:I_LEN] for t in range(PAIRS)]
            wvt_ps = [u_ps[t][0:I_LEN, 128:256] for t in range(PAIRS)]

            # --- input DMAs: w/s0/ident first (iter-0 capsule step needs
            # only these), x/xt on other queues in parallel.
            nc.scalar.dma_start(out=wpack_sb[0:64], in_=wpack_d[:])
            nc.scalar.dma_start(out=s_sb[:], in_=s0_d[:])
            nc.scalar.dma_start(out=ident[:], in_=ident_d[:])
            nc.scalar.dma_start(out=wpack_sb[64:128], in_=wpack_sb[0:64])
            nc.sync.dma_start(out=x_sb[:], in_=x_nat_d[:])
            nc.gpsimd.dma_start(out=xt_sb[:], in_=xt_d[:])
            nc.vector.memset(shift[:], -40.0)

            for r in range(3):
                # --- softmax + s-step (iters 1,2; iter 0 s comes from host)
                if r > 0:
                    for b in range(B):
                        t, b2 = divmod(b, 2)
                        # exp(l - 40): softmax-invariant shift keeps exp and
                        # 1/Z in range (logits span [-86, 92] here).
                        nc.scalar.activation(
                            out=pexp[:, b], in_=logits_ps[b // 2][:, b % 2],
                            func=Exp, bias=shift[:],
                        )
                        with nc.allow_low_precision(reason="bf16 softmax Z"):
                            (nc.gpsimd if CFG_ZGPS else nc.vector).reduce_sum(
                                out=zsum[:, b], in_=pexp[:, b], axis=X
                            )
                        with nc.allow_low_precision(reason="bf16 softmax 1/Z"):
                            nc.vector.reciprocal(out=rinv[:, b], in_=zsum[:, b])
                        (nc.vector if CFG_XRDVE else nc.gpsimd).tensor_mul(
                            out=xr[:, b],
                            in0=x_sb[:, b],
                            in1=rinv[:, b].unsqueeze(-1).broadcast_to((128, C, I_LEN)),
                        )
                        # s[o,i] = sum_n probs * x
                        for c in range(C):
                            nc.tensor.matmul(
                                out=s_ps[t][b2 * 64 : (b2 + 1) * 64, :],
                                lhsT=pexp[:, b, c, :],
                                rhs=xr[:, b, c, :],
                                start=(c == 0),
                                stop=(c == C - 1),
                                tile_position=(0, 64 * b2),
                            )
                    for t in range(PAIRS):
                        nc.scalar.copy(out=s_sb[:, t, :], in_=s_ps[t][:])

                # --- out-step: v_raw[o,l] = sum_i W[o,l,i] * s[o,i], bf16
                for t in range(PAIRS):
                    nc.vector.tensor_mul(
                        out=prod[:, t],
                        in0=wli_sb[:],
                        in1=s_sb[:, t, :].unsqueeze(1).broadcast_to((128, L_LEN, I_LEN)),
                    )
                    with nc.allow_low_precision(reason="bf16 capsule out"):
                        nc.vector.reduce_sum(out=v_raw[:, t, :], in_=prod[:, t], axis=X)

                # --- squash factor chain (pair-batched, small ops), runs in
                # parallel with the wv_raw contraction below.
                nc.vector.tensor_mul(out=sq[:], in0=v_raw[:], in1=v_raw[:])
                nc.vector.reduce_sum(out=ns[:], in_=sq[:], axis=X)
                nc.scalar.activation(out=lnns[:], in_=ns[:], func=Ln)
                nc.scalar.activation(out=vnorm[:], in_=lnns[:], func=Exp, scale=0.5)
                nc.vector.tensor_scalar_add(out=denom[:], in0=ns[:], scalar1=1.0)
                nc.vector.reciprocal(out=rden[:], in_=denom[:])

                if r == 2:
                    # final: v = (v_raw * ||v||) * (1/(1+||v||^2)), fp32 out
                    for t in range(PAIRS):
                        nc.vector.scalar_tensor_tensor(
                            out=v_out[:, t],
                            in0=v_raw[:, t],
                            scalar=vnorm[:, t : t + 1],
                            in1=rden[:, t : t + 1].broadcast_to((128, L_LEN)),
                            op0=MUL,
                            op1=MUL,
                        )
                    nc.sync.dma_start(out=out_d[:], in_=v_out[:])
                else:
                    nc.vector.tensor_mul(out=fsc[:], in0=vnorm[:], in1=rden[:])
                    # wv_raw[o,i] = sum_l W[o,l,i] * v_raw[o,l]  (unscaled)
                    for t in range(PAIRS):
                        nc.vector.tensor_mul(
                            out=wprod[:, t],
                            in0=wil_sb[:],
                            in1=v_raw[:, t, :]
                            .unsqueeze(1)
                            .broadcast_to((128, I_LEN, L_LEN)),
                        )
                        with nc.allow_low_precision(reason="bf16 capsule wv"):
                            nc.vector.reduce_sum(
                                out=wv_raw[:, t, :], in_=wprod[:, t], axis=X
                            )
                        # wv = f * wv_raw (squash factor commutes through the
                        # linear W contraction)
                        nc.vector.tensor_scalar_mul(
                            out=wv_f[:, t, :],
                            in0=wv_raw[:, t, :],
                            scalar1=fsc[:, t : t + 1],
                        )
                        nc.tensor.transpose(
                            out=wvt_ps[t][:], in_=wv_f[:, t, :], identity=ident[:]
                        )
                        nc.scalar.copy(out=wvt_sb[:, t, :], in_=wvt_ps[t][:])
                    # logits[n,o] += sum_i x[n,i] * wv[o,i]
                    # r0: one start/stop per 2KB psum bank (8 chunks per bank).
                    # r1: accumulate onto surviving has_written bits; the sim's
                    # group bookkeeping can't express re-opening, so skip it.
                    for b in range(B):
                        t, b2 = divmod(b, 2)
                        for c in range(C):
                            k = (b % 2) * C + c
                            nc.tensor.matmul(
                                out=logits_ps[b // 2][:, b % 2, c, :],
                                lhsT=xt_sb[:, b, c, :],
                                rhs=wvt_sb[:, t, b2 * 64 : (b2 + 1) * 64],
                                start=(r == 0 and k % 8 == 0),
                                stop=(r == 0 and (k % 8 == 7 or k == 2 * C - 1)),
                                skip_group_check=(r == 1),
                            )
    return nc


_NC = None


def get_nc():
    global _NC
    if _NC is None:
        _NC = build_nc()
    return _NC


def to_bf16(a):
    import ml_dtypes

    return a.astype(ml_dtypes.bfloat16)


def make_in_maps(x, weight):
    x = np.ascontiguousarray(x, dtype=np.float32)
    w = np.ascontiguousarray(weight, dtype=np.float32)
    # rows 0..63: [wli flat | wil flat]
    wpack = np.concatenate(
        [w.reshape(O_CAPS, L_LEN * I_LEN), w.transpose(0, 2, 1).reshape(O_CAPS, -1)],
        axis=1,
    )
    ident = np.eye(128, dtype=np.float32)
    in_maps = []
    for core in range(NCORES):
        xs = x[core * B : (core + 1) * B]  # [B, 1152, 32]
        xc = xs.reshape(B, C, 128, I_LEN)
        x_nat = np.ascontiguousarray(xc.transpose(2, 0, 1, 3))  # [128, B, C, 32]
        xt = np.ascontiguousarray(xc.transpose(3, 0, 1, 2))  # [32, B, C, 128]
        # iteration-0 s: probs uniform (1/64) -> s0[b,i] = sum_n x[b,n,i]/64,
        # replicated across the 64 o-partitions of each half.
        s0 = xs.sum(axis=1) / O_CAPS  # [B, 32]
        # s_sb[b2*64+o, t, i] = s0[2t+b2, i]
        s0_tile = np.empty((2, O_CAPS, PAIRS, I_LEN), dtype=np.float32)
        for b2 in range(2):
            for t in range(PAIRS):
                s0_tile[b2, :, t, :] = s0[2 * t + b2]
        s0_tile = s0_tile.reshape(128, PAIRS, I_LEN)
        in_maps.append(
            {
                "x_nat": to_bf16(x_nat),
                "xt": to_bf16(xt),
                "wpack": to_bf16(wpack),
                "ident": ident,
                "s0": to_bf16(s0_tile),
            }
        )
    return in_maps


def assemble(results):
    outs = []
    for core in range(len(results)):
        o = np.asarray(results[core]["out"], dtype=np.float32)  # [128, PAIRS, 32]
        # v[b2*64+o, t, l] -> [b=2t+b2, o, l]
        o = o.reshape(2, O_CAPS, PAIRS, L_LEN).transpose(2, 0, 1, 3)  # [t, b2, o, l]
        outs.append(o.reshape(B, O_CAPS, L_LEN))
    return np.concatenate(outs, axis=0)


def _pin_act_table_set(nc):
    """Make Exp and Ln resolve to the one table set containing both
    (natural_log_exp_and_others), so the whole kernel runs on a single
    ACT table load instead of thrashing 1.3us loads between exp/ln sets."""
    from concourse.hw_specs import get_activation_tables

    tabs = get_activation_tables(nc.m.arch)
    for name, funcs in tabs.items():
        if name != "natural_log_exp_and_others":
            funcs.discard(Exp)
            funcs.discard(Ln)
            funcs.discard(mybir.ActivationFunctionType.Square)
            funcs.discard(mybir.ActivationFunctionType.Copy)
            funcs.discard(mybir.ActivationFunctionType.Identity)


def run(x, weight, trace=False):
    nc = get_nc()
    if not nc.is_finalized():
        _pin_act_table_set(nc)
        nc.finalize()  # run Bacc lowering passes (wait splitting, reg alloc)
    res = run_bass_kernel_spmd(nc, make_in_maps(x, weight), list(range(NCORES)), trace=trace)
    return assemble(res.results), res


def kernel(x, weight):
    out, _ = run(x, weight)
    return out


# revision 13
# speedup vs baseline: 1.3944x; 1.2036x over previous
"""CapsuleLinear (dynamic routing) Trainium2 kernel, v3.

Reference computes priors = einsum('oli,bni->bonl', W, x) (302MB) then runs 3
routing iterations. We never materialize priors. Key algebraic identity: the
logits update needs wv = W^T(squash(W s)) = f * (W^T W) s = f * G s with
f = ||Ws||/(1+||Ws||^2) and ns = ||Ws||^2 = <s, G s>.  G[o] = W[o]^T W[o] is a
static per-capsule 32x32 matrix, precomputed on the host.  So iterations 0-1
never form v at all; only the final iteration applies W itself:

  per routing iteration r:
    probs[n,o]   = softmax_o(logits[n,o])          (exp on ACT, Z on DVE/GPS)
    s[o,i]       = sum_n probs[n,o] x[n,i]         (PE matmul, contract n)
    q[o,i]       = sum_i' G[o,i,i'] s[o,i']        (DVE/GPS bf16 mul+reduce)
    ns[o]        = sum_i s[o,i] q[o,i]             (tiny)
    wv[o,i]      = f(ns) * q[o,i]                  (tiny scale)
    logits[n,o] += sum_i x[n,i] wv[o,i]            (PE matmul, contract i)
  final r: v = squash(W s) via wli, DMA out.

Iteration 0 has uniform probs -> s0 = colsum(x)/64, so wv0 depends only on
colmean(x) and W: computed on the host (~0.5% of FLOPs) and shipped as a 16KB
wvT tile.  Device iteration 0 is just the 36 delta matmuls.

Sharding: data-parallel over batch N=32 -> 4 batches per core on 8 cores.
Weight (64,32,32) replicated. No collectives.

sqrt(ns) is computed as exp(0.5*ln(ns)) so the whole kernel uses one ACT
table set (natural_log_exp_and_others) - no 1.3us table switches.

Per-core layouts:
  x_sb  [128(p), 4(b), 9(c), 32(i)]   x[b, c*128+p, i]          bf16
  xt_sb [32(i), 4(b), 9(c), 128(p)]   host-transposed x         bf16
  gw    [128(b2*64+o), 2, 32, 32]     [G | wli] pair-replicated bf16
  logits PSUM [128(p), 2(b2), 9(c), 64(o)] x2 halves, fp32
  pair tiles [128(b2*64+o), 2(pair), ...] 2 batches stacked on partitions
Engine split: pair-0 reduces + Z(b0,b1) + xr(b2,b3) on DVE; pair-1 reduces +
Z(b2,b3) + xr(b0,b1) on GPSIMD; exp/copies/ln/exp on ACT.
"""

import os
import sys

for _p in ("/opt/trn_rl_repo",):
    if _p not in sys.path and os.path.isdir(_p):
        sys.path.insert(0, _p)

import numpy as np

import concourse.bacc as bacc
import concourse.bass as bass
import concourse.tile as tile
from concourse import mybir
from concourse.bass_utils import run_bass_kernel_spmd

N_TOT, N_CAPS, I_LEN = 32, 1152, 32
O_CAPS, L_LEN = 64, 32
NCORES = 8
B = N_TOT // NCORES  # 4 batches per core
C = N_CAPS // 128    # 9 chunks of 128 input capsules
PAIRS = B // 2
FP = mybir.dt.float32
BF = mybir.dt.bfloat16
Exp = mybir.ActivationFunctionType.Exp
Ln = mybir.ActivationFunctionType.Ln
X = mybir.AxisListType.X
MUL = mybir.AluOpType.mult


def build_nc():
    nc = bacc.Bacc("TRN2", target_bir_lowering=False, debug=True)
    x_nat_d = nc.dram_tensor("x_nat", [128, B, C, I_LEN], BF, kind="ExternalInput")
    xt_d = nc.dram_tensor("xt", [I_LEN, B, C, 128], BF, kind="ExternalInput")
    # [G | wli], both pair-replicated to 128 rows on the host (no on-chip
    # replication chain on the critical path)
    gw_d = nc.dram_tensor("gw", [128, 2, L_LEN, I_LEN], BF, kind="ExternalInput")
    ident_d = nc.dram_tensor("ident", [128, 128], FP, kind="ExternalInput")
    wvt0_d = nc.dram_tensor("wvt0", [I_LEN, PAIRS, 128], BF, kind="ExternalInput")
    out_d = nc.dram_tensor("out", [128, PAIRS, L_LEN], FP, kind="ExternalOutput")

    with tile.TileContext(nc) as tc:
        with (
            tc.tile_pool(name="main", bufs=1) as pool,
            tc.tile_pool(name="psum", bufs=1, space="PSUM") as psum,
        ):
            x_sb = pool.tile([128, B, C, I_LEN], BF)
            xt_sb = pool.tile([I_LEN, B, C, 128], BF)
            ph = pool.tile([128, B, C, O_CAPS // 2], BF)
            gw_sb = pool.tile([128, 2, L_LEN, I_LEN], BF)
            g_sb = gw_sb[:, 0]            # [128, 32(i), 32(i')]
            wli_sb = gw_sb[:, 1]          # [128, 32(l), 32(i)]
            ident = pool.tile([128, 128], FP)
            shift = pool.tile([128, 1], FP)
            s_sb = pool.tile([128, PAIRS, I_LEN], BF)
            pexp = pool.tile([128, B, C, O_CAPS], BF)
            zsum = pool.tile([128, B, C], BF)
            rinv = pool.tile([128, B, C], BF)
            xr = pool.tile([128, B, C, I_LEN], BF)
            qprod = pool.tile([128, PAIRS, L_LEN, I_LEN], BF)
            q_raw = pool.tile([128, PAIRS, I_LEN], BF)
            nsprod = pool.tile([128, PAIRS, I_LEN], BF)
            ns = pool.tile([128, PAIRS], FP)
            lnns = pool.tile([128, PAIRS], FP)
            vnorm = pool.tile([128, PAIRS], FP)
            denom = pool.tile([128, PAIRS], FP)
            rden = pool.tile([128, PAIRS], FP)
            fsc = pool.tile([128, PAIRS], FP)
            wv_f = pool.tile([128, PAIRS, I_LEN], FP)
            wvt_sb = pool.tile([I_LEN, PAIRS, 128], BF)
            v_raw = pool.tile([128, PAIRS, L_LEN], BF)
            v_out = pool.tile([128, PAIRS, L_LEN], FP)

            logits_ps = [
                psum.tile([128, 2, C, O_CAPS], FP, name=f"logits_ps{h}", tag=f"lg{h}")
                for h in range(2)
            ]
            u_ps = [
                psum.tile([128, 512], FP, name=f"u_ps{t}", tag=f"u_ps{t}")
                for t in range(PAIRS)
            ]
            s_ps = [u_ps[t][:, 0:I_LEN] for t in range(PAIRS)]
            wvt_ps = [u_ps[t][0:I_LEN, 128:256] for t in range(PAIRS)]

            # --- input DMAs. gpsimd queue: wvt0 (16KB, feeds iter-0 delta)
            # then xt (needed right after) then x. scalar queue: gw + ident
            # (needed from iter 1 on).
            nc.gpsimd.dma_start(out=wvt_sb[:], in_=wvt0_d[:])
            nc.sync.dma_start(out=xt_sb[:], in_=xt_d[:])
            nc.scalar.dma_start(out=gw_sb[:], in_=gw_d[:])
            nc.sync.dma_start(out=x_sb[:], in_=x_nat_d[:])
            nc.scalar.dma_start(out=ident[:], in_=ident_d[:])
            nc.vector.memset(shift[:], -40.0)

            for r in range(3):
                if r > 0:
                    # --- softmax over o (free dim) + s matmuls, per batch.
                    # Z on DVE for b0/b1, GPS for b2/b3; xr the other way.
                    for b in range(B):
                        t, b2 = divmod(b, 2)
                        nc.scalar.activation(
                            out=pexp[:, b], in_=logits_ps[b // 2][:, b % 2],
                            func=Exp, bias=shift[:],
                        )
                        # Z in two stages: o-fold 64->32 on GPS (TT runs
                        # there), the halved reduce + 1/Z on DVE.
                        xeng = nc.gpsimd if b < 2 else nc.vector
                        nc.gpsimd.tensor_add(
                            out=ph[:, b],
                            in0=pexp[:, b, :, 0:32],
                            in1=pexp[:, b, :, 32:64],
                        )
                        with nc.allow_low_precision(reason="bf16 softmax"):
                            nc.vector.reduce_sum(out=zsum[:, b], in_=ph[:, b], axis=X)
                            nc.vector.reciprocal(out=rinv[:, b], in_=zsum[:, b])
                        xeng.tensor_mul(
                            out=xr[:, b],
                            in0=x_sb[:, b],
                            in1=rinv[:, b].unsqueeze(-1).broadcast_to((128, C, I_LEN)),
                        )
                        for c in range(C):
                            nc.tensor.matmul(
                                out=s_ps[t][b2 * 64 : (b2 + 1) * 64, :],
                                lhsT=pexp[:, b, c, :],
                                rhs=xr[:, b, c, :],
                                start=(c == 0),
                                stop=(c == C - 1),
                                tile_position=(0, 64 * b2),
                            )
                    for t in range(PAIRS):
                        nc.scalar.copy(out=s_sb[:, t, :], in_=s_ps[t][:])

                if r == 2:
                    # --- final: v = squash(W s), fp32 out, one DMA.
                    nc.vector.tensor_mul(
                        out=qprod[:],
                        in0=wli_sb[:].unsqueeze(1).broadcast_to((128, PAIRS, L_LEN, I_LEN)),
                        in1=s_sb[:].unsqueeze(2).broadcast_to((128, PAIRS, L_LEN, I_LEN)),
                    )
                    with nc.allow_low_precision(reason="bf16 capsule out"):
                        nc.vector.reduce_sum(out=v_raw[:], in_=qprod[:], axis=X)
                    nc.vector.tensor_mul(out=nsprod[:], in0=v_raw[:], in1=v_raw[:])
                    nc.vector.reduce_sum(out=ns[:], in_=nsprod[:], axis=X)
                    nc.scalar.activation(out=lnns[:], in_=ns[:], func=Ln)
                    nc.scalar.activation(out=vnorm[:], in_=lnns[:], func=Exp, scale=0.5)
                    nc.vector.tensor_scalar_add(out=denom[:], in0=ns[:], scalar1=1.0)
                    nc.vector.reciprocal(out=rden[:], in_=denom[:])
                    for t in range(PAIRS):
                        nc.vector.scalar_tensor_tensor(
                            out=v_out[:, t],
                            in0=v_raw[:, t],
                            scalar=vnorm[:, t : t + 1],
                            in1=rden[:, t : t + 1].broadcast_to((128, L_LEN)),
                            op0=MUL,
                            op1=MUL,
                        )
                    nc.sync.dma_start(out=out_d[:], in_=v_out[:])
                elif r > 0:
                    # --- q = G s; ns = <s,q>; wv = f(ns) * q.  Pair 0 reduce
                    # on DVE, pair 1 on GPS; factor chain batched over pairs.
                    nc.vector.tensor_mul(
                        out=qprod[:],
                        in0=g_sb[:].unsqueeze(1).broadcast_to((128, PAIRS, I_LEN, I_LEN)),
                        in1=s_sb[:].unsqueeze(2).broadcast_to((128, PAIRS, I_LEN, I_LEN)),
                    )
                    with nc.allow_low_precision(reason="bf16 capsule q"):
                        nc.vector.reduce_sum(out=q_raw[:], in_=qprod[:], axis=X)
                    nc.vector.tensor_mul(out=nsprod[:], in0=s_sb[:], in1=q_raw[:])
                    nc.vector.reduce_sum(out=ns[:], in_=nsprod[:], axis=X)
                    nc.scalar.activation(out=lnns[:], in_=ns[:], func=Ln)
                    nc.scalar.activation(out=vnorm[:], in_=lnns[:], func=Exp, scale=0.5)
                    nc.vector.tensor_scalar_add(out=denom[:], in0=ns[:], scalar1=1.0)
                    nc.vector.reciprocal(out=rden[:], in_=denom[:])
                    nc.vector.tensor_mul(out=fsc[:], in0=vnorm[:], in1=rden[:])
                    nc.vector.tensor_mul(
                        out=wv_f[:],
                        in0=q_raw[:],
                        in1=fsc[:].unsqueeze(-1).broadcast_to((128, PAIRS, I_LEN)),
                    )
                    for t in range(PAIRS):
                        nc.tensor.transpose(
                            out=wvt_ps[t][:], in_=wv_f[:, t, :], identity=ident[:]
                        )
                        nc.scalar.copy(out=wvt_sb[:, t, :], in_=wvt_ps[t][:])

                if r < 2:
                    # logits[n,o] += sum_i x[n,i] * wv[o,i]
                    # r0: one start/stop per 2KB psum bank (8 chunks per bank).
                    # r1: accumulate onto surviving has_written bits.
                    for b in range(B):
                        t, b2 = divmod(b, 2)
                        for c in range(C):
                            k = (b % 2) * C + c
                            nc.tensor.matmul(
                                out=logits_ps[b // 2][:, b % 2, c, :],
                                lhsT=xt_sb[:, b, c, :],
                                rhs=wvt_sb[:, t, b2 * 64 : (b2 + 1) * 64],
                                start=(r == 0 and k % 8 == 0),
                                stop=(r == 0 and (k % 8 == 7 or k == 2 * C - 1)),
                                skip_group_check=(r == 1),
                            )
    return nc


_NC = None


def get_nc():
    global _NC
    if _NC is None:
        _NC = build_nc()
    return _NC


def to_bf16(a):
    import ml_dtypes

    return a.astype(ml_dtypes.bfloat16)


def make_in_maps(x, weight):
    x = np.ascontiguousarray(x, dtype=np.float32)
    w = np.ascontiguousarray(weight, dtype=np.float32)  # [64, 32(l), 32(i)]
    G = np.einsum("oli,olj->oij", w, w)  # [64, 32(i), 32(i')]
    gw = np.stack([G, w], axis=1)  # [64, 2, 32, 32]
    gw = np.tile(gw, (2, 1, 1, 1))  # pair-replicated [128, 2, 32, 32]
    ident = np.eye(128, dtype=np.float32)
    in_maps = []
    for core in range(NCORES):
        xs = x[core * B : (core + 1) * B]  # [B, 1152, 32]
        xc = xs.reshape(B, C, 128, I_LEN)
        x_nat = np.ascontiguousarray(xc.transpose(2, 0, 1, 3))  # [128, B, C, 32]
        xt = np.ascontiguousarray(xc.transpose(3, 0, 1, 2))  # [32, B, C, 128]
        # iteration-0: probs uniform -> s0 = colsum(x)/64; wv0 = f0 * G s0
        s0 = xs.sum(axis=1) / O_CAPS  # [B, 32]
        q0 = np.einsum("oij,bj->boi", G, s0)  # [B, 64, 32]
        ns0 = np.einsum("boi,bi->bo", q0, s0)  # [B, 64]
        f0 = np.sqrt(ns0) / (1.0 + ns0)
        wv0 = f0[:, :, None] * q0  # [B, 64, 32]
        # wvt0[i, t, b2*64+o] = wv0[2t+b2, o, i]
        wvt0 = np.empty((I_LEN, PAIRS, 2, O_CAPS), dtype=np.float32)
        for t in range(PAIRS):
            for b2 in range(2):
                wvt0[:, t, b2, :] = wv0[2 * t + b2].T
        wvt0 = wvt0.reshape(I_LEN, PAIRS, 128)
        in_maps.append(
            {
                "x_nat": to_bf16(x_nat),
                "xt": to_bf16(xt),
                "gw": to_bf16(gw),
                "ident": ident,
                "wvt0": to_bf16(wvt0),
            }
        )
    return in_maps


def assemble(results):
    outs = []
    for core in range(len(results)):
        o = np.asarray(results[core]["out"], dtype=np.float32)  # [128, PAIRS, 32]
        # v[b2*64+o, t, l] -> [b=2t+b2, o, l]
        o = o.reshape(2, O_CAPS, PAIRS, L_LEN).transpose(2, 0, 1, 3)  # [t, b2, o, l]
        outs.append(o.reshape(B, O_CAPS, L_LEN))
    return np.concatenate(outs, axis=0)


def _pin_act_table_set(nc):
    """Make Exp and Ln resolve to the one table set containing both
    (natural_log_exp_and_others), so the whole kernel runs on a single
    ACT table load."""
    from concourse.hw_specs import get_activation_tables

    tabs = get_activation_tables(nc.m.arch)
    for name, funcs in tabs.items():
        if name != "natural_log_exp_and_others":
            funcs.discard(Exp)
            funcs.discard(Ln)
            funcs.discard(mybir.ActivationFunctionType.Square)
            funcs.discard(mybir.ActivationFunctionType.Copy)
            funcs.discard(mybir.ActivationFunctionType.Identity)


def run(x, weight, trace=False):
    nc = get_nc()
    if not nc.is_finalized():
        _pin_act_table_set(nc)
        nc.finalize()
    res = run_bass_kernel_spmd(nc, make_in_maps(x, weight), list(range(NCORES)), trace=trace)
    return assemble(res.results), res


def kernel(x, weight):
    out, _ = run(x, weight)
    return out
